# revision 20
# baseline (speedup 1.0000x reference)
"""Trainium2 Bass kernel for nn_Attention (B=4, N=2048, C=1024, H=16 heads).

Sharding: head-parallel x batch — core c handles batch c//2 and the 8 heads
(c%2)*8..(c%2)*8+7 (Megatron-style split of qkv output dim / proj input dim).
Each core emits 4 head-pair partial projections [N, C]; host sums 8 partials
per batch (2 cores x 4 pairs) plus the folded bias.

Software-pipelined per-head-pair schedule (all matmuls float32r):
  v:     once upfront, natural [t, f] layout, with a ones column per head
         => the A@V matmul also yields softmax denominators for free.
  loop:  attention(g) [ACT-heavy] ; qkv+rope(g+1) [DVE-heavy] ;
         norm(g) ; proj(g)  — so RoPE for the next pair overlaps the
         current pair's softmax work across engines.
  qkv:   q^T,k^T in [f,t] layout; RoPE fused with bias via DVE
         scalar_tensor_tensor reading PSUM; attention scale D^-0.5 split
         sqrt-wise into the shared cos/sin tables.
  attn:  S^T[j,i] = kR_h^T.T @ qR_h (K=64); exp on ACT over 2-bank PSUM
         tiles (no max subtraction -- logits ~N(0,1)); O^T = [V_h|1]^T @ A^T.
  norm:  per-pair batched DVE reciprocal; partition-broadcast of recip rows
         via one-hot matmul; in-place DVE multiply.
  proj:  Y_g[i,c] = O_pair^T.T @ Wp[pair rows] (K=128 single-shot MMs),
         PSUM->SBUF copies on DVE, DMA out.
"""
import sys

sys.path.insert(0, "/opt/trn_rl_repo")

import numpy as np

B, N, C = 4, 2048, 1024
H, D = 16, 64
HPC, FPC = 8, 512     # heads / features per core
P = 128

_CACHE = {}


def _build_nc():
    import concourse.bass as bass
    import concourse.bacc as bacc
    import concourse.mybir as mybir
    import concourse.tile as tile

    dt = mybir.dt
    f32 = dt.float32
    f32r = dt.float32r
    AF = mybir.ActivationFunctionType
    ALU = mybir.AluOpType
    PSUM = bass.MemorySpace.PSUM

    nc = bacc.Bacc("TRN2", target_bir_lowering=False, debug=False, num_devices=8)

    xt = nc.dram_tensor("xt", [C, N], f32r, kind="ExternalInput").ap()
    wq = nc.dram_tensor("wq", [C, FPC], f32r, kind="ExternalInput").ap()
    wk = nc.dram_tensor("wk", [C, FPC], f32r, kind="ExternalInput").ap()
    wv = nc.dram_tensor("wv", [C, FPC], f32r, kind="ExternalInput").ap()
    wp = nc.dram_tensor("wp", [FPC, C], f32r, kind="ExternalInput").ap()
    bq = nc.dram_tensor("bq", [P, 4], f32, kind="ExternalInput").ap()
    bk = nc.dram_tensor("bk", [P, 4], f32, kind="ExternalInput").ap()
    coss = nc.dram_tensor("coss", [P, N], f32, kind="ExternalInput").ap()
    sinss = nc.dram_tensor("sinss", [P, N], f32, kind="ExternalInput").ap()
    sel8 = nc.dram_tensor("sel8", [P, 256], f32r, kind="ExternalInput").ap()
    onesb = nc.dram_tensor("onesb", [P, 1024], f32r, kind="ExternalInput").ap()
    y4 = nc.dram_tensor("y4", [4, N, C], f32, kind="ExternalOutput").ap()

    with tile.TileContext(nc) as tc:
        with (
            tc.tile_pool(name="small", bufs=1) as small,
            tc.tile_pool(name="ropec", bufs=1) as ropec,
            tc.tile_pool(name="qrk", bufs=2) as qrk,
            tc.tile_pool(name="vaug", bufs=1) as vaugp,
            tc.tile_pool(name="xts", bufs=2) as xtp,
            tc.tile_pool(name="wqk", bufs=1) as wqkp,
            tc.tile_pool(name="scr", bufs=1) as scr,
            tc.tile_pool(name="opair", bufs=2) as opp,
            tc.tile_pool(name="atp", bufs=2) as atp,
            tc.tile_pool(name="rcp", bufs=1) as rcp,
            tc.tile_pool(name="wpp", bufs=1) as wpp,
            tc.tile_pool(name="yst", bufs=2) as yst,
            tc.tile_pool(name="ps_mm", bufs=2, space=PSUM) as psa,
            tc.tile_pool(name="ps_st", bufs=2, space=PSUM) as pst,
            tc.tile_pool(name="ps_ot", bufs=2, space=PSUM) as pso,
        ):
            sel_sb = small.tile([P, 256], f32r)
            ones_sb = small.tile([P, 1024], f32r)
            bq_sb = small.tile([P, 4], f32)
            bk_sb = small.tile([P, 4], f32)
            cos_sb = ropec.tile([P, N], f32)
            sin_sb = ropec.tile([P, N], f32)
            v_aug = vaugp.tile([P, 16, HPC, 65], f32r)
            nc.sync.dma_start(bq_sb[:], bq)
            nc.sync.dma_start(bk_sb[:], bk)
            nc.sync.dma_start(cos_sb[:], coss)
            nc.sync.dma_start(sin_sb[:], sinss)
            nc.sync.dma_start(ones_sb[:], onesb)
            nc.sync.dma_start(sel_sb[:], sel8)
            nc.vector.tensor_copy(v_aug[:, :, :, 64:65], ones_sb[:, 0:P])

            def load_wv(wvp):
                wv_sb = [wvp.tile([P, FPC], f32r, tag=f"wv{ct}",
                                  name=f"wvs{ct}") for ct in range(8)]
                for ct in range(8):
                    nc.sync.dma_start(wv_sb[ct][:], wv[ct * P:(ct + 1) * P, :])
                return wv_sb

            def qkrope(g, qt, kt, wv_sb=None):
                # hoisted per-pair weight slices
                wqc = [wqkp.tile([P, P], f32r, tag=f"wqc{ct}",
                                 name=f"wqc{ct}") for ct in range(8)]
                wkc = [wqkp.tile([P, P], f32r, tag=f"wkc{ct}",
                                 name=f"wkc{ct}") for ct in range(8)]
                for ct in range(8):
                    nc.sync.dma_start(
                        wqc[ct][:], wq[ct * P:(ct + 1) * P, g * P:(g + 1) * P])
                    nc.sync.dma_start(
                        wkc[ct][:], wk[ct * P:(ct + 1) * P, g * P:(g + 1) * P])
                for tc_i in range(4):
                    xts = [xtp.tile([P, 512], f32r, tag=f"xts{ct}",
                                    name=f"xq{ct}") for ct in range(8)]
                    for ct in range(8):
                        nc.sync.dma_start(
                            xts[ct][:], xt[ct * P:(ct + 1) * P,
                                           tc_i * 512:(tc_i + 1) * 512])
                    for (dst, wsb, bsb) in (
                        (qt, wqc, bq_sb), (kt, wkc, bk_sb),
                    ):
                        ps = psa.tile([P, 512], f32, tag="mm", name="psqk")
                        for ct in range(8):
                            nc.tensor.matmul(
                                ps[:], wsb[ct][:], xts[ct][:],
                                start=(ct == 0), stop=(ct == 7),
                            )
                        cosc = cos_sb[:, tc_i * 512:(tc_i + 1) * 512]
                        sinc = sin_sb[:, tc_i * 512:(tc_i + 1) * 512]
                        out = dst[:, tc_i * 512:(tc_i + 1) * 512]
                        tmp = scr.tile([P, 512], f32, tag="tmp", name="tmp")
                        u = scr.tile([P, 512], f32, tag="u", name="u")
                        nc.vector.scalar_tensor_tensor(
                            tmp[:], ps[:], bsb[:, g:g + 1], cosc,
                            ALU.add, ALU.mult)
                        for blk in range(4):
                            r0 = blk * 32
                            s0 = r0 + 32 if blk % 2 == 0 else r0 - 32
                            nc.vector.scalar_tensor_tensor(
                                u[r0:r0 + 32, :], ps[s0:s0 + 32, :],
                                bsb[s0:s0 + 32, g:g + 1],
                                sinc[s0:s0 + 32, :],
                                ALU.add, ALU.mult)
                        nc.vector.tensor_add(out, tmp[:], u[:])
                    if wv_sb is not None:
                        # v shares this x pass (pair 0 only), after qk so
                        # rope feeds the first attention ASAP
                        for tt in range(4):
                            jt = tc_i * 4 + tt
                            ps = psa.tile([P, 512], f32, tag="mm", name="psv")
                            for ct in range(8):
                                nc.tensor.matmul(
                                    ps[:],
                                    xts[ct][:, tt * P:(tt + 1) * P],
                                    wv_sb[ct][:],
                                    start=(ct == 0), stop=(ct == 7),
                                )
                            nc.vector.tensor_copy(v_aug[:, jt, :, 0:64], ps[:])

            def attn_half(g, hl, qt, kt, op):
                h = 2 * g + hl
                p0 = hl * 64
                steps = [(ic, jp) for ic in range(4) for jp in range(8)]
                ots, sts = {}, {}

                def emit_S(k):
                    ic, jp = steps[k]
                    if jp == 0:
                        ots[ic] = pso.tile([65, 512], f32, tag="ot",
                                           name="ot")
                    st = pst.tile([P, 1024], f32, tag="st", name="st")
                    sts[k] = st
                    i0 = ic * 512
                    for sub in range(2):
                        jt = 2 * jp + sub
                        nc.tensor.matmul(
                            st[:, sub * 512:(sub + 1) * 512],
                            kt[p0:p0 + 64, jt * P:(jt + 1) * P],
                            qt[p0:p0 + 64, i0:i0 + 512],
                        )

                # S-MMs run one step ahead of exp/O-MMs so the PE never
                # serializes exp(k+1) behind O(k) at chunk boundaries
                emit_S(0)
                for k in range(32):
                    ic, jp = steps[k]
                    i0 = ic * 512
                    st = sts.pop(k)
                    at = atp.tile([P, 1024], f32r, tag="at", name="at")
                    nc.scalar.activation(at[:], st[:], AF.Exp)
                    if k + 1 < 32:
                        emit_S(k + 1)
                    for sub in range(2):
                        jt = 2 * jp + sub
                        nc.tensor.matmul(
                            ots[ic][:],
                            v_aug[:, jt, h, 0:65],
                            at[:, sub * 512:(sub + 1) * 512],
                            start=(jt == 0), stop=(jt == 15),
                        )
                    if jp == 7:
                        ot = ots.pop(ic)
                        # high priority: these free the ot PSUM slot; don't
                        # let them queue behind bulk rope/proj DVE work
                        with tc.high_priority():
                            nc.vector.tensor_copy(
                                op[p0:p0 + 64, i0:i0 + 512], ot[0:64, :])
                            nc.vector.tensor_copy(
                                op[32 * ic:32 * ic + 1,
                                   2048 + hl * 512:2048 + (hl + 1) * 512],
                                ot[64:65, :])

            def recip_half(hl, op, rc):
                # reciprocal of this half's denominators; emitted right after
                # the half's attention so it overlaps the other half on DVE
                with nc.allow_low_precision(reason="f32r for matmul"):
                    nc.vector.reciprocal(
                        rc[:, hl * 512:(hl + 1) * 512],
                        op[:, 2048 + hl * 512:2048 + (hl + 1) * 512])

            def norm_half(hl, op, rc):
                p0 = hl * 64
                for ic in range(4):
                    i0 = ic * 512
                    bc = pso.tile([64, 512], f32, tag="ot", name="psbc")
                    nc.tensor.matmul(
                        bc[:],
                        sel_sb[:, ic * 64:(ic + 1) * 64],
                        rc[:, hl * 512:(hl + 1) * 512],
                    )
                    nc.vector.tensor_mul(
                        op[p0:p0 + 64, i0:i0 + 512],
                        op[p0:p0 + 64, i0:i0 + 512],
                        bc[:])

            def proj(g, op):
                wps = wpp.tile([P, C], f32r, tag="wps", name="wps")
                nc.sync.dma_start(wps[:], wp[g * P:(g + 1) * P, :])
                for it in range(16):
                    yt = yst.tile([P, C], f32, tag="yt", name="yt")
                    for cc in range(2):
                        ps = psa.tile([P, 512], f32, tag="mm", name="psy")
                        nc.tensor.matmul(
                            ps[:],
                            op[:, it * P:(it + 1) * P],
                            wps[:, cc * 512:(cc + 1) * 512],
                        )
                        # ACT is idle after the last pair's exps; use it
                        if g == 3 and cc == 0:
                            nc.scalar.copy(yt[:, cc * 512:(cc + 1) * 512],
                                           ps[:])
                        else:
                            nc.vector.tensor_copy(
                                yt[:, cc * 512:(cc + 1) * 512], ps[:])
                    nc.sync.dma_start(
                        y4[g, it * P:(it + 1) * P, :], yt[:])

            # software pipeline: rope(g+1) is emitted BETWEEN the two
            # attention halves of pair g so the DVE ropes while ACT exps;
            # v shares pair 0's x pass on the PE
            qts, kts = {}, {}
            qts[0] = qrk.tile([P, N], f32r, tag="qR", name="qR0")
            kts[0] = qrk.tile([P, N], f32r, tag="kR", name="kR0")
            with tc.tile_pool(name="wvp", bufs=1) as wvp:
                wv_sb = load_wv(wvp)
                qkrope(0, qts[0], kts[0], wv_sb=wv_sb)
            for g in range(4):
                op = opp.tile([P, 3072], f32r, tag="op", name="op")
                nc.vector.tensor_copy(op[:, 2048:3072], ones_sb[:])
                rc = rcp.tile([P, 1024], f32r, tag="rc", name="rc")
                attn_half(g, 0, qts[g], kts[g], op)
                recip_half(0, op, rc)
                if g < 3:
                    qts[g + 1] = qrk.tile([P, N], f32r, tag="qR",
                                          name=f"qR{g + 1}")
                    kts[g + 1] = qrk.tile([P, N], f32r, tag="kR",
                                          name=f"kR{g + 1}")
                    qkrope(g + 1, qts[g + 1], kts[g + 1])
                attn_half(g, 1, qts[g], kts[g], op)
                recip_half(1, op, rc)
                norm_half(0, op, rc)
                norm_half(1, op, rc)
                proj(g, op)

    nc.compile()
    return nc


def host_prep(x, w_qkv, b_qkv, w_proj, b_proj):
    inv_freq = 1.0 / (10000.0 ** (np.arange(0, D, 2, dtype=np.float32) / D))
    t = np.arange(N, dtype=np.float32)
    freqs = np.outer(t, inv_freq).astype(np.float32)
    cosL = np.cos(freqs).T.astype(np.float32)
    sinL = np.sin(freqs).T.astype(np.float32)
    # attention scale D^-0.5 split sqrt-wise onto the q and k rope tables
    rs = np.float32(D ** -0.25)
    cosT = np.ascontiguousarray(np.tile(cosL, (4, 1)) * rs)
    # rows aligned to the rotate-half *source* rows (DVE requires equal
    # input base partitions): out[r0:r0+32] = q[s0:s0+32] * sins[s0:s0+32]
    sinS = np.ascontiguousarray(
        np.concatenate([sinL, -sinL, sinL, -sinL], axis=0) * rs)
    sel = np.zeros((P, 256), dtype=np.float32)
    for ic in range(4):
        sel[32 * ic, ic * 64:(ic + 1) * 64] = 1.0
    maps = []
    for c in range(8):
        b, hg = c // 2, c % 2
        f0 = hg * FPC
        maps.append({
            "xt": np.ascontiguousarray(np.asarray(x)[b].T),
            "wq": np.ascontiguousarray(w_qkv[:, f0:f0 + FPC]),
            "wk": np.ascontiguousarray(w_qkv[:, C + f0:C + f0 + FPC]),
            "wv": np.ascontiguousarray(w_qkv[:, 2 * C + f0:2 * C + f0 + FPC]),
            "wp": np.ascontiguousarray(w_proj[f0:f0 + FPC, :]),
            "bq": np.ascontiguousarray(b_qkv[f0:f0 + FPC].reshape(4, P).T),
            "bk": np.ascontiguousarray(b_qkv[C + f0:C + f0 + FPC].reshape(4, P).T),
            "coss": cosT,
            "sinss": sinS,
            "sel8": sel,
            "onesb": np.ones((P, 1024), np.float32),
        })
    return maps


def kernel(x, w_qkv, b_qkv, w_proj, b_proj):
    from concourse.bass_utils import run_bass_kernel_spmd

    x = np.asarray(x, dtype=np.float32)
    w_qkv = np.asarray(w_qkv, dtype=np.float32)
    b_qkv = np.asarray(b_qkv, dtype=np.float32)
    w_proj = np.asarray(w_proj, dtype=np.float32)
    b_proj = np.asarray(b_proj, dtype=np.float32)

    if "nc" not in _CACHE:
        _CACHE["nc"] = _build_nc()
    nc = _CACHE["nc"]

    maps = host_prep(x, w_qkv, b_qkv, w_proj, b_proj)
    res = run_bass_kernel_spmd(nc, maps, list(range(8))).results
    b_eff = (b_proj + b_qkv[2 * C:] @ w_proj).astype(np.float32)
    out = np.empty((B, N, C), np.float32)
    for b in range(B):
        acc = res[2 * b]["y4"].sum(axis=0) + res[2 * b + 1]["y4"].sum(axis=0)
        out[b] = acc + b_eff
    return out
